# revision 46
# baseline (speedup 1.0000x reference)
"""Trainium2 Bass kernel for nn_Attention_81776177315877.

Separable-conv attention block (CMT/PVT style):
  x (B=8, 3136, 256) -> q/k/v = sepconv(dw3x3+BN+pw1x1, k/v stride 2)
  -> 8-head attention (d=32) -> proj.

Sharding: data-parallel over batch, core b <- batch b. No collectives.

Device strategy (per core):
  - depthwise 3x3 (with BN scale folded in) as 9 per-channel
    multiply-accumulates on DVE (q, k) and GpSimd (v) over a zero-padded
    channel-major image; pointwise 1x1 as full-K matmuls on the tensor
    engine (BN shift + dw bias folded into a per-channel constant).
  - attention: S^T (keys on partitions) per-head matmuls (K=32), exp on
    ScalarE split into two half-tiles per key tile so ScalarE never
    stalls on PSUM reuse; O^T via lhsT = [V_h | ones*32] (64 cols) which
    replicates the softmax denominator onto 32 extra rows for free; O
    matmuls are emitted one key-tile late so they never block the PE
    queue on exp.  Normalization: full-tile reciprocal + partition-
    misaligned multiplies on DVE.
  - projection consumes the pair-layout directly with host-permuted
    proj weights, producing token-major output tiles DMA'd straight to
    DRAM.  q-pointwise/proj work is interleaved into the attention loop
    to keep the tensor engine continuously busy (p-state!).
"""

import sys

sys.path.insert(0, "/opt/trn_rl_repo")

import numpy as np
import ml_dtypes

import concourse.bass as bass
import concourse.bacc as bacc
import concourse.mybir as mybir
import concourse.tile as tile
from concourse.bass_utils import run_bass_kernel_spmd

FP = mybir.dt.float32
BF = mybir.dt.bfloat16
AF = mybir.ActivationFunctionType
MUL = mybir.AluOpType.mult
ADD = mybir.AluOpType.add

C = 256
HEADS = 8
D = 32
HH = 56
N = HH * HH          # 3136 query tokens
HK = 28
NK = HK * HK         # 784 key tokens
PADW = HH + 2        # 58
EPS = 1e-5
SCALE = D ** -0.5

IC_CH = 8            # query rows per chunk -> 448 free
IC_F = IC_CH * HH    # 448
N_IC = HH // IC_CH   # 7
JT = 112             # key tile (partitions) for attention
N_JT = NK // JT      # 7
QT = 112             # proj query tile

_CACHED = {}


def _build_nc():
    nc = bacc.Bacc("TRN2", target_bir_lowering=False, debug=False, num_devices=8)

    xpad_d = nc.dram_tensor("xpad", [C, PADW * PADW], BF, kind="ExternalInput")
    xph_d = nc.dram_tensor("xph", [C, 4 * 29 * 29], BF, kind="ExternalInput")
    dw9_d = {}
    const_d = {}
    pwt_d = {}
    for p in ("q", "k", "v"):
        dw9_d[p] = nc.dram_tensor(f"{p}_dw9", [2, 128, 9], FP, kind="ExternalInput")
        const_d[p] = nc.dram_tensor(f"{p}_const", [C, 1], FP, kind="ExternalInput")
        pwt_d[p] = nc.dram_tensor(f"{p}_pwt", [C, C], BF, kind="ExternalInput")
    pwtp_d = nc.dram_tensor("pwt_pairs", [2, 128, C], BF, kind="ExternalInput")
    pbb_d = nc.dram_tensor("pb_bcast", [128, C], FP, kind="ExternalInput")
    out_d = nc.dram_tensor("out", [N, C], FP, kind="ExternalOutput")

    with tile.TileContext(nc) as tc:
        with (
            tc.tile_pool(name="persist", bufs=1) as pp,
            tc.tile_pool(name="ep", bufs=2) as ep,
            tc.tile_pool(name="rp", bufs=2) as rp,
            tc.tile_pool(name="osb", bufs=2) as osbp,
            tc.tile_pool(name="outp", bufs=3) as outp,
            tc.tile_pool(name="pss", bufs=1, space="PSUM") as pss,      # S halves
            tc.tile_pool(name="pso", bufs=1, space="PSUM") as pso,      # O pairs
            tc.tile_pool(name="psw", bufs=2, space="PSUM") as psw,      # pw/proj/transpose
        ):
            from concourse.masks import make_identity

            ident = pp.tile([128, 128], FP, tag="ident", name="ident")
            make_identity(nc, ident[:])

            # ---- load weights ----
            dw9 = {}
            consts = {}
            pwt = {}
            for p in ("q", "k", "v"):
                dw9[p] = [pp.tile([128, 9], FP, tag=f"dw9_{p}{cb}", name=f"dw9_{p}{cb}") for cb in range(2)]
                consts[p] = [pp.tile([128, 1], FP, tag=f"const_{p}{cb}", name=f"const_{p}{cb}") for cb in range(2)]
                pwt[p] = [pp.tile([128, C], BF, tag=f"pwt_{p}{cb}", name=f"pwt_{p}{cb}") for cb in range(2)]
                for cb in range(2):
                    nc.sync.dma_start(dw9[p][cb][:], dw9_d[p][cb])
                    nc.sync.dma_start(consts[p][cb][:], const_d[p][cb * 128:(cb + 1) * 128, :])
                    nc.sync.dma_start(pwt[p][cb][:], pwt_d[p][cb * 128:(cb + 1) * 128, :])
            pwtp = [pp.tile([128, C], BF, tag=f"pwtp{i}", name=f"pwtp{i}") for i in range(2)]
            for i in range(2):
                nc.sync.dma_start(pwtp[i][:], pwtp_d[i])
            pbb = pp.tile([128, C], FP, tag="pbb", name="pbb")
            nc.sync.dma_start(pbb[:], pbb_d[:, :])

            # ---- phase 0: host-padded channel-major images, flat DMAs ----
            x_pad = [pp.tile([128, PADW * PADW], BF, tag=f"xpad{cb}", name=f"xpad{cb}") for cb in range(2)]
            xph = [pp.tile([128, 4 * 29 * 29], BF, tag=f"xph{cb}", name=f"xph{cb}") for cb in range(2)]
            for cb in range(2):
                nc.sync.dma_start(x_pad[cb][:], xpad_d[cb * 128:(cb + 1) * 128, :])
                nc.sync.dma_start(xph[cb][:], xph_d[cb * 128:(cb + 1) * 128, :])

            # ---- depthwise helpers ----
            def shifted(cb, tap, stride, r_out0, nrows, wo):
                dh, dw = tap // 3 - 1, tap % 3 - 1
                if stride == 1:
                    r0 = 1 + r_out0 + dh
                    c0 = 1 + dw
                    xpv = x_pad[cb][:].rearrange("p (h w) -> p h w", w=PADW)
                    return xpv[:, r0:r0 + nrows, c0:c0 + wo]
                # stride 2: padded coords (1+2i+dh, 1+2j+dw) via phase images
                pr, ro = (1 + dh) % 2, (1 + dh) // 2
                pc, co = (1 + dw) % 2, (1 + dw) // 2
                xpp = xph[cb][:].rearrange(
                    "p (a b h w) -> p (a b) h w", a=2, b=2, w=29
                )
                return xpp[
                    :, 2 * pr + pc, ro + r_out0:ro + r_out0 + nrows, co:co + wo
                ]

            def dw_chunk(eng, p, dst, cb, stride, r_out0, nrows, wo,
                         taps=range(9)):
                # depthwise accum into dst[cb] rows [r_out0, r_out0+nrows)
                dv = dst[cb][:].rearrange("p (r w) -> p r w", w=wo)[
                    :, r_out0:r_out0 + nrows, :
                ]
                for tap in taps:
                    if tap == 0:
                        eng.tensor_scalar(
                            dv, shifted(cb, 0, stride, r_out0, nrows, wo),
                            dw9[p][cb][:, 0:1], None, MUL,
                        )
                    else:
                        eng.scalar_tensor_tensor(
                            dv, shifted(cb, tap, stride, r_out0, nrows, wo),
                            dw9[p][cb][:, tap:tap + 1], dv, MUL, ADD,
                        )

            def pw_mm(p, dwt, dst, csl, fsz):
                # pointwise 1x1: dst[cbo][:, csl] = pw @ dw + const
                for cbo in range(2):
                    cps = psw.tile([128, 448], FP, tag="w", name="w")
                    for cbi in range(2):
                        nc.tensor.matmul(
                            cps[:, :fsz],
                            lhsT=(pwt[p][cbi][:, cbo * 128:(cbo + 1) * 128]),
                            rhs=(dwt[cbi][:, csl]),
                            start=(cbi == 0),
                            stop=(cbi == 1),
                            skip_group_check=True,
                        )
                    nc.vector.tensor_scalar_add(
                        dst[cbo][:, csl], cps[:, :fsz], consts[p][cbo]
                    )

            # ---- k conv up front; v conv pipelined per key tile ----
            dw_k = [pp.tile([128, NK], BF, tag=f"dwk{cb}", name=f"dwk{cb}") for cb in range(2)]
            dw_v = [pp.tile([128, NK], BF, tag=f"dwv{cb}", name=f"dwv{cb}") for cb in range(2)]
            k_cm = [pp.tile([128, NK], BF, tag=f"kcm{cb}", name=f"kcm{cb}") for cb in range(2)]
            v_cm = [pp.tile([128, NK], FP, tag=f"vcm{cb}", name=f"vcm{cb}") for cb in range(2)]
            for cb in range(2):
                dw_chunk(nc.vector, "k", dw_k, cb, 2, 0, HK, HK)
            for ch in range(2):
                pw_mm("k", dw_k, k_cm, slice(ch * 392, (ch + 1) * 392), 392)

            # v64[:, jt, h, 0:32] = V_h token-major; [..., 32:64] = 1.0 so the
            # O matmul emits the softmax denominator replicated on 32 rows
            v64 = pp.tile([JT, N_JT, HEADS, 64], BF, tag="v64", name="v64")
            nc.gpsimd.memset(v64[:, :, :, 32:64], 1.0)

            def v_unit(jt):
                def run():
                    for cb in range(2):
                        dw_chunk(nc.vector, "v", dw_v, cb, 2, 4 * jt, 4, HK)
                    pw_mm("v", dw_v, v_cm, slice(jt * JT, (jt + 1) * JT), JT)
                    for cb in range(2):
                        tp = psw.tile([128, 448], FP, tag="w", name="w")
                        nc.tensor.transpose(
                            tp[:JT, :128],
                            v_cm[cb][:, jt * JT:(jt + 1) * JT],
                            ident[:],
                        )
                        nc.vector.tensor_copy(
                            v64[:JT, jt, cb * 4:(cb + 1) * 4, 0:32],
                            tp[:JT, :128].rearrange("p (h d) -> p h d", d=32),
                        )
                return run

            # ---- attention state tiles ----
            dw_q = [pp.tile([128, N], BF, tag=f"dwq{cb}", name=f"dwq{cb}") for cb in range(2)]
            q_cm = [pp.tile([128, N], BF, tag=f"qcm{cb}", name=f"qcm{cb}") for cb in range(2)]
            # O-pair accumulators: head A rows 0..32 + its denom on 32..64,
            # head B rows 64..96 + its denom on 96..128 (fully written)
            op_t = [pso.tile([128, 448], FP, tag=f"op{t}", name=f"op{t}") for t in range(2)]


            def q_conv_units(ic):
                units = []
                csl = slice(ic * IC_F, (ic + 1) * IC_F)
                for cb in range(2):
                    for t0 in range(0, 9, 3):
                        def mk_dw(cb, t0):
                            def run():
                                dw_chunk(nc.vector, "q", dw_q, cb, 1,
                                         ic * IC_CH, IC_CH, HH,
                                         taps=range(t0, t0 + 3))
                            return run
                        units.append(mk_dw(cb, t0))
                units.append(lambda: pw_mm("q", dw_q, q_cm, csl, IC_F))
                return units

            # q conv for ic=0 runs up front
            for u in q_conv_units(0):
                u()

            def proj_units(ic, osb_tiles):
                """Token-major projection for query chunk ic (4 qtiles)."""
                units = []
                for qt in range(4):
                    def mk(qt):
                        def run():
                            pjt = psw.tile([128, 448], FP, tag="w", name="w")
                            pj = pjt[:QT, :C]
                            for i in range(2):
                                nc.tensor.matmul(
                                    pj,
                                    lhsT=(osb_tiles[i][:, qt * QT:(qt + 1) * QT]),
                                    rhs=(pwtp[i][:]),
                                    start=(i == 0),
                                    stop=(i == 1),
                                    skip_group_check=True,
                                )
                            ot = outp.tile([QT, C], FP, tag="ot", name="ot")
                            nc.vector.tensor_add(ot[:], pj, pbb[:QT, :])
                            nc.sync.dma_start(
                                out_d[(ic * 4 + qt) * QT:(ic * 4 + qt + 1) * QT, :],
                                ot[:],
                            )
                        return run
                    units.append(mk(qt))
                return units

            # ---- attention: one global stream of (ic, hg, jt) windows with
            # carried O/normalize emission so ScalarE never sees a boundary ----
            op_t_ref = op_t
            pending = []          # closures to emit right after this window's S/exp
            pending2 = []         # one window later
            osb_by_ic = {}
            bg = []

            def make_norm(hg, ic):
                osb = osbp.tile([128, IC_F], BF, tag=f"osb{hg}", name=f"osb{hg}")

                def mk(t):
                    def run():
                        r = rp.tile([128, IC_F], FP, tag=f"r{t}", name=f"r{t}")
                        nc.vector.reciprocal_approx_fast(r[:], op_t[t][:])
                        nc.vector.tensor_mul(
                            osb[32 * t:32 * t + 32, :], op_t[t][0:32, :], r[32:64, :]
                        )
                        nc.vector.tensor_mul(
                            osb[64 + 32 * t:96 + 32 * t, :], op_t[t][64:96, :], r[96:128, :]
                        )
                    return run
                return osb, mk(0), mk(1)

            def emit_o(ic, hg, jt, e_pair):
                for half in range(2):
                    for j in range(2):
                        h = hg * 4 + 2 * half + j
                        nc.tensor.matmul(
                            op_t[half][64 * j:64 * j + 64, :],
                            lhsT=(v64[:JT, jt, h, :]),
                            rhs=(e_pair[half][:, j, :]),
                            start=(jt == 0),
                            stop=(jt == N_JT - 1),
                            tile_position=(0, 64 * j),
                            skip_group_check=True,
                        )

            windows = [(ic, hg, jt) for ic in range(N_IC) for hg in range(2)
                       for jt in range(N_JT)]
            for (ic, hg, jt) in windows:
                if hg == 0 and jt == 0:
                    # refresh background queue for this ic
                    if ic == 0:
                        bg += [v_unit(j) for j in range(N_JT)]
                    if ic + 1 < N_IC:
                        bg += q_conv_units(ic + 1)
                    if ic - 1 in osb_by_ic:
                        bg += proj_units(ic - 1, osb_by_ic.pop(ic - 1))

                s_t = [pss.tile([JT, 2, 512], FP, tag=f"s{h}", name=f"s{h}") for h in range(2)]
                e_t = [None, None]
                for half in range(2):
                    for j in range(2):
                        hh = 2 * half + j
                        nc.tensor.matmul(
                            s_t[half][:, j, :IC_F],
                            lhsT=(k_cm[hg][hh * 32:(hh + 1) * 32, jt * JT:(jt + 1) * JT]),
                            rhs=(q_cm[hg][hh * 32:(hh + 1) * 32, ic * IC_F:(ic + 1) * IC_F]),
                            start=True,
                            stop=True,
                            tile_position=(32 * hh, 0),
                        )
                    e_t[half] = ep.tile([JT, 2, IC_F], BF, tag=f"e{half}", name=f"e{half}")
                    nc.scalar.activation(
                        e_t[half][:, :, :], s_t[half][:, :, :IC_F], AF.Exp, scale=SCALE
                    )
                # carried work: previous window's O matmuls (+ normalize)
                for w in pending:
                    w()
                pending = pending2 + [lambda ic=ic, hg=hg, jt=jt, e=e_t: emit_o(ic, hg, jt, e)]
                pending2 = []
                if jt == N_JT - 1:
                    osb, nr0, nr1 = make_norm(hg, ic)
                    osb_by_ic.setdefault(ic, []).append(osb)
                    pending.append(nr0)
                    pending2.append(nr1)
                # one background unit per window, except the head-group
                # boundary window (carried O + normalize own the engines
                # there).  ic0 stays ungated: its 14 units need all 14 slots,
                # and gating would spill writers past their readers.
                if bg and not (jt == 0 and ic > 0):
                    bg.pop(0)()
            for w in pending + pending2:
                w()
            while bg:
                bg.pop(0)()
            for u in proj_units(N_IC - 1, osb_by_ic.pop(N_IC - 1)):
                u()

    nc.compile()
    return nc


def _host_inputs(inp):
    common = {}
    for p in ("q", "k", "v"):
        scale = inp[f"{p}_bn_g"] / np.sqrt(inp[f"{p}_bn_v"] + EPS)
        shift = inp[f"{p}_bn_b"] - inp[f"{p}_bn_m"] * scale
        dw9 = (inp[f"{p}_dw_w"].reshape(C, 9) * scale[:, None]).astype(np.float32)
        common[f"{p}_dw9"] = np.ascontiguousarray(dw9.reshape(2, 128, 9))
        common[f"{p}_pwt"] = np.ascontiguousarray(
            inp[f"{p}_pw_w"].T
        ).astype(ml_dtypes.bfloat16)
        const = (
            inp[f"{p}_pw_w"] @ (scale * inp[f"{p}_dw_b"] + shift) + inp[f"{p}_pw_b"]
        ).astype(np.float32)
        common[f"{p}_const"] = const.reshape(C, 1)
    # packed-pair projection weights: osb tile of head group hg holds head
    # channels in row order [4hg, 4hg+2, 4hg+1, 4hg+3] (32 rows each)
    wt = np.ascontiguousarray(inp["proj_w"].T).astype(np.float32)  # (c, o)
    pwtp = np.zeros((2, 128, C), np.float32)
    for hg in range(2):
        for slot, h in enumerate((4 * hg, 4 * hg + 2, 4 * hg + 1, 4 * hg + 3)):
            pwtp[hg, 32 * slot:32 * slot + 32] = wt[32 * h:32 * h + 32]
    common["pwt_pairs"] = pwtp.astype(ml_dtypes.bfloat16)
    common["pb_bcast"] = np.ascontiguousarray(
        np.tile(inp["proj_b"].reshape(1, C), (128, 1))
    ).astype(np.float32)
    return common


def _in_maps(inp):
    common = _host_inputs(inp)
    xb = np.asarray(inp["x"]).astype(np.float32).astype(ml_dtypes.bfloat16)
    B = xb.shape[0]
    xcm = xb.transpose(0, 2, 1)                      # (B, C, N)
    xp = np.zeros((B, C, PADW, PADW), xb.dtype)
    xp[:, :, 1:57, 1:57] = xcm.reshape(B, C, HH, HH)
    xph = np.ascontiguousarray(
        xp.reshape(B, C, 29, 2, 29, 2).transpose(0, 1, 3, 5, 2, 4)
    ).reshape(B, C, 4 * 29 * 29)
    xpf = np.ascontiguousarray(xp.reshape(B, C, PADW * PADW))
    return [
        dict(common, xpad=xpf[b], xph=xph[b]) for b in range(B)
    ]


def kernel(**inputs):
    inp = {k: np.asarray(v) for k, v in inputs.items()}
    B = inp["x"].shape[0]

    if "nc" not in _CACHED:
        _CACHED["nc"] = _build_nc()
    nc = _CACHED["nc"]

    in_maps = _in_maps(inp)
    res = run_bass_kernel_spmd(nc, in_maps, list(range(B)))
    out = np.stack([res.results[b]["out"] for b in range(B)], axis=0)
    return out.astype(np.float32)
